# revision 1
# baseline (speedup 1.0000x reference)
"""Grouped channel self-interaction kernel for Trainium2 (8 NeuronCores).

out[b, c] = inp[b, c] * (sum of inp[b, c'] over c' in c's group of 8) / 32

Input [32, 256, 56, 56] f32. Sharding: data-parallel over batch, 4 batches
per core. Per core the slice is viewed as [128, 8, 3136]: partition rows are
(batch, group) pairs (4*32 = 128 exactly), free axis is (channel-in-group,
spatial). Every partition row is fully contiguous in DRAM.

All compute runs on VectorE: 7 adds build the group sum per spatial chunk,
then 8 scalar_tensor_tensor ops compute (x * 1/32) * group_sum. Single-engine
compute keeps every instruction at <=1 semaphore wait (walrus codegen limit).
"""

import numpy as np

_B, _C, _H, _W = 32, 256, 56, 56
_S = _H * _W              # 3136
_NCORES = 8
_BPC = _B // _NCORES      # 4 batches per core
_G = 32                   # groups
_CPG = 8                  # channels per group
_SCALE = 1.0 / 32.0       # 1 / NUM_GROUPS

_CHUNK = 784              # spatial columns per tile
_NCHUNK = _S // _CHUNK    # 4: deeper DMA/compute/store pipeline; Bacc legalizes sync waits

_cache: dict = {}


def _build_nc():
    import concourse.bacc as bacc
    import concourse.mybir as mybir
    from concourse.tile import TileContext

    f32 = mybir.dt.float32
    mult = mybir.AluOpType.mult
    # Bacc (not raw Bass): its compile() runs generate_event_semaphores(),
    # which splits sync waits to satisfy the 1-wait-per-instruction HW limit.
    nc = bacc.Bacc()
    x = nc.dram_tensor("inp", [128, _CPG, _S], f32, kind="ExternalInput")
    y = nc.dram_tensor("out", [128, _CPG, _S], f32, kind="ExternalOutput")

    with TileContext(nc) as tc:
        with (
            tc.tile_pool(name="xin", bufs=_NCHUNK) as xpool,
            # acc lives in PSUM (otherwise unused): bufs=4 makes each chunk's
            # accumulator a fresh tile, so the first add of a chunk carries
            # only the input-DMA wait (no same-engine WAR wait on top).
            tc.tile_pool(name="acc", bufs=_NCHUNK, space="PSUM") as apool,
            tc.tile_pool(name="yout", bufs=_NCHUNK) as opool,
        ):
            for k in range(_NCHUNK):
                sl = slice(k * _CHUNK, (k + 1) * _CHUNK)
                # One buffer per chunk (no slot reuse): in-DMAs then carry no
                # WAR/WAW waits, out-DMAs read a tile whose only writer is
                # DVE — every instruction stays at <=1 sync wait (walrus cap).
                xt = xpool.tile([128, _CPG, _CHUNK], f32)
                nc.sync.dma_start(xt[:], x[:, :, sl])
                acc = apool.tile([128, _CHUNK], f32)
                nc.vector.tensor_add(acc[:], xt[:, 0, :], xt[:, 1, :])
                for c in range(2, _CPG):
                    nc.vector.tensor_add(acc[:], acc[:], xt[:, c, :])
                ot = opool.tile([128, _CPG, _CHUNK], f32)
                for c in range(_CPG):
                    nc.vector.scalar_tensor_tensor(
                        ot[:, c, :], xt[:, c, :], _SCALE, acc[:], mult, mult
                    )
                nc.sync.dma_start(y[:, :, sl], ot[:])
    nc.compile()
    return nc


def _in_maps(inp: np.ndarray) -> list:
    x = np.ascontiguousarray(inp, dtype=np.float32).reshape(
        _NCORES, _BPC * _G, _CPG, _S
    )
    return [{"inp": x[i]} for i in range(_NCORES)]


def kernel(inp: np.ndarray) -> np.ndarray:
    from concourse.bass_utils import run_bass_kernel_spmd

    if "nc" not in _cache:
        _cache["nc"] = _build_nc()
    res = run_bass_kernel_spmd(_cache["nc"], _in_maps(inp), list(range(_NCORES)))
    out = np.stack([np.asarray(res.results[i]["out"]) for i in range(_NCORES)])
    return out.reshape(_B, _C, _H, _W)

